# revision 32
# baseline (speedup 1.0000x reference)
"""BGE-M3 sparse-embedding head (matvec + relu + scatter-max into (B, V))
as a Bass/Tile kernel on 8 Trainium2 NeuronCores.

Sharding: data-parallel over batch; each core computes 4 of 32 rows.
Each row is processed as TWO half-row pipeline units (512 tokens each);
a unit covers a disjoint half of the dense output row, since tokens are
host-assigned to band-chunks by their dense column f = v % 1954 and the
8 fixed ~244-col bands split exactly at 977.

Per unit (4 chunks x 128 token slots):
  1. matvec on the PE: host uploads hidden TRANSPOSED+permuted fp16
     ([partition h%128, h//128, token-slot]); lhsT = w-block replicated
     across 128 columns, rhs = x^T -> every psum partition of ptw[128,512]
     holds the unit's tw vector. Act copies psum row 0 to SBUF; 4 tiny PE
     transposes land tw token-major in spare psd columns; DVE applies
     relu(tw + b) reading PSUM directly.
  2. DVE builds one fused route one-hot akh[slot, k, p] (p = v // 1954, or
     fixup class q, or -1) and, per chunk, rk[slot, 0:246|246:251] =
     onehot(off) * tw covering both the band columns and 5 fixup member
     slots.
  3. PE assembles the half-dense [128, 977] at column offset DOFF (24/23,
     so every band is PSUM-bank-clean) with ONE matmul per band
     (disjoint -> start=stop=True, no zeroing pass) + one 5-col fixup
     matmul per chunk sharing the same lhsT (self-contained start/stop:
     open accumulation groups interleaved with same-bank matmuls get
     corrupted). Act casts PSUM -> fp16 (DVE for the last unit, so the
     final two evacuations overlap), DMA writes the half.
  4. Duplicate ids / band-overflow tokens go to <=128 fixup classes per
     row (members may ride in any chunk of either half): members bucket
     into psum cols [1004, 1024), per-half reduce_max then a cross-half
     max -> per-class value, scattered by ONE 128-index indirect DMA
     ordered after the row's dense writes. Unused class slots target
     index V and are dropped by the bounds check.
DMA scheduling exploits two trace-measured facts: queues execute DMAs
serially with ~3-4us pipeline latency each (so wrep rides the early
SWDGE queue, unit 0's input leads the sync queue, and _throttle_prefetch
gates bulk prefetch on the first matmul's own waits), and back-to-back
matmuls pipeline at ~0.42ns/col with LDWEIGHTS fully hidden.
Output is written fp16 (250112-padded) and upcast to f32 on the host.
"""

import numpy as np

import concourse.bass as bass
import concourse.mybir as mybir
import concourse.tile as tile
from concourse.bass import IndirectOffsetOnAxis
from concourse.bass_utils import run_bass_kernel_spmd

V = 250002
NCORES = 8
B, L, H = 32, 1024, 1024
BS = B // NCORES            # batch rows per core (4)
P = 128
W = 1954                    # dense row width per partition (128*1954 >= V)
VPAD = P * W                # 250112
NB = 8                      # h-blocks / band-chunks per row
MAXCLS = P                  # fixup classes per row
MAXMEM = 5                  # member slots per (fixup class, chunk)
FIXC0 = 246                 # rk column where fixup member slots start
MEMMARK = 2000.0            # iwc value marking member slot m: MEMMARK + m
RKW = FIXC0 + MAXMEM        # 251
F32 = mybir.dt.float32
F16 = mybir.dt.float16
I32 = mybir.dt.int32

# fixed band boundaries over [0, W)
BB = [round(W * k / NB) for k in range(NB + 1)]   # [0,244,488,733,977,1221,1466,1710,1954]
PSUM_BANK = 512  # f32 elements per PSUM bank; matmul out must not cross banks
DW = W // 2      # dense columns per half-row unit (977)
TW0 = 0          # tw transpose columns in the half psd tile
PSF0 = 1004      # fixup member region in the half psd tile (4 chunks x 5)
# dense offset per half, chosen so no band crosses a 512-col PSUM bank bound
DOFF = [24, 23]


def _half_bands(h):
    """Per band kk of half h: (lo, hi) psd-tile column range (bank-clean)."""
    off = h * DW
    out = []
    for kk in range(4):
        lo, hi = BB[h * 4 + kk] - off + DOFF[h], BB[h * 4 + kk + 1] - off + DOFF[h]
        assert (lo // PSUM_BANK) == ((hi - 1) // PSUM_BANK), (h, kk, lo, hi)
        out.append((lo, hi))
    return out


BANDS_H = [_half_bands(0), _half_bands(1)]

_MAX_WAITS = 1


def _split_excess_waits(nc, cap=_MAX_WAITS):
    """walrus's gen3 codegen rejects >1 sync-wait per instruction; move the
    excess onto NoOps inserted just before (same engine => order kept)."""
    n = 0
    for func in nc.m.functions:
        for bb in func.blocks:
            newlist = []
            for ins in bb.instructions:
                si = getattr(ins, "sync_info", None)
                if si is not None and si.on_wait and len(si.on_wait) > cap:
                    waits = list(si.on_wait)
                    extra, keep = waits[:-cap], waits[-cap:]
                    while extra:
                        chunk, extra = extra[:cap], extra[cap:]
                        nop = mybir.InstNoOp(
                            name=f"{ins.name}-wsplit-{n}", ins=[], outs=[]
                        )
                        nop.engine = ins.engine
                        nop.sync_info = mybir.SyncInfo(on_wait=chunk, on_update=[])
                        newlist.append(nop)
                        n += 1
                    ins.sync_info = mybir.SyncInfo(
                        on_wait=keep, on_update=list(si.on_update)
                    )
                newlist.append(ins)
            bb.instructions = newlist
    return n


def _throttle_prefetch(nc):
    """Delay the bulk input prefetches in(2)/in(4) (3rd/4th sync-queue DMA
    triggers) until unit 0's input DMA completes: a queue's DMA engines
    fair-share packets across all outstanding transfers, so without this the
    first matvec's input finishes ~5us late behind bulk prefetch traffic."""
    insts = [i for bb in nc.m.functions[0].blocks for i in bb.instructions]
    mm = next(i for i in insts if isinstance(i, mybir.InstMatmult))
    if not (mm.sync_info and mm.sync_info.on_wait):
        return
    gates = list(mm.sync_info.on_wait)
    sp_dmas = [
        i for i in insts
        if isinstance(i, mybir.InstDMACopy) and str(i.engine) == "EngineType.SP"
    ]
    for idx in (1, 2):  # in(2), in(4): sync dmas after the unit-0 fine half
        if idx < len(sp_dmas):
            ins = sp_dmas[idx]
            si = getattr(ins, "sync_info", None)
            if si is None or not si.on_wait:
                ins.sync_info = mybir.SyncInfo(
                    on_wait=list(gates),
                    on_update=list(si.on_update) if si else [],
                )


def _build_program():
    nc = bass.Bass()
    Op = mybir.AluOpType

    xt2 = nc.declare_dram_parameter("xt2", [BS * P, NB * L], F16, isOutput=False)
    wrep = nc.declare_dram_parameter("wrep", [P, NB * P], F16, isOutput=False)
    ipt = nc.declare_dram_parameter("ipt", [P, 4 * P], F16, isOutput=False)
    iwc = nc.declare_dram_parameter("iwc", [P, RKW], F32, isOutput=False)
    bcol = nc.declare_dram_parameter("bcol", [P, 1], F32, isOutput=False)
    route = nc.declare_dram_parameter("route", [P, BS * NB], F16, isOutput=False)
    offrel = nc.declare_dram_parameter("offrel", [P, BS * NB], F32, isOutput=False)
    fixgid = nc.declare_dram_parameter("fixgid", [P, BS], I32, isOutput=False)
    outs = [
        nc.declare_dram_parameter(f"out{r}", [VPAD], F16, isOutput=True)
        for r in range(BS)
    ]

    NU = 2 * BS  # half-row pipeline units

    with tile.TileContext(nc) as tc:
        with (
            tc.tile_pool(name="pers", bufs=1) as pers,
            tc.tile_pool(name="xt", bufs=8) as xt_tp,
            tc.tile_pool(name="akh", bufs=3) as akh_tp,
            tc.tile_pool(name="rk", bufs=5) as rk_tp,
            tc.tile_pool(name="tw", bufs=3) as tw_tp,
            tc.tile_pool(name="twrow", bufs=3) as twrow_tp,
            tc.tile_pool(name="dense", bufs=4) as dense_tp,
            tc.tile_pool(name="fv", bufs=3) as fv_tp,
            tc.tile_pool(name="fixv", bufs=2) as fixv_tp,
            tc.tile_pool(name="ptw", bufs=2, space="PSUM") as ptw_tp,
            tc.tile_pool(name="psd", bufs=3, space="PSUM") as psd_tp,
        ):
            # ---- wrep first on the gpsimd queue (its SWDGE starts earliest,
            # and keeping it off sync lets unit 0's input be the sync queue's
            # first transfer: each queue adds ~3-4us of per-DMA latency) ----
            wrep_t = pers.tile([P, NB * P], F16, tag="wrep")
            nc.gpsimd.dma_start(out=wrep_t[:], in_=wrep[:])

            # ---- remaining constants on the gpsimd queue ----
            ipt_t = pers.tile([P, 4 * P], F16, tag="ipt")
            nc.gpsimd.dma_start(out=ipt_t[:], in_=ipt[:])
            iwc_t = pers.tile([P, RKW], F32, tag="iwc")
            nc.gpsimd.dma_start(out=iwc_t[:], in_=iwc[:])
            bcol_t = pers.tile([P, 1], F32, tag="bcol")
            nc.gpsimd.dma_start(out=bcol_t[:], in_=bcol[:])
            route_t = pers.tile([P, BS * NB], F16, tag="route")
            nc.gpsimd.dma_start(out=route_t[:], in_=route[:])
            offr_t = pers.tile([P, BS * NB], F32, tag="offr")
            nc.gpsimd.dma_start(out=offr_t[:], in_=offrel[:])
            fg_t = pers.tile([P, BS], I32, tag="fg")
            nc.gpsimd.dma_start(out=fg_t[:], in_=fixgid[:])

            ones1_t = pers.tile([1, 1], F32, tag="ones1")
            nc.vector.memset(ones1_t[:], 1.0)

            xt_tiles = {}

            def emit_in(u, fine=False):
                # unit u = (row r, half h): contiguous 8KB per partition
                r, h = divmod(u, 2)
                xt_t = xt_tp.tile([P, 4 * L], F16, tag="xt")
                base = h * 4 * L
                if fine:
                    for i, eng in enumerate((nc.sync, nc.scalar)):
                        eng.dma_start(
                            out=xt_t[:, i * 2 * L : (i + 1) * 2 * L],
                            in_=xt2[r * P : (r + 1) * P,
                                    base + i * 2 * L : base + (i + 1) * 2 * L],
                        )
                else:
                    eng = nc.sync if u % 2 == 0 else nc.scalar
                    eng.dma_start(
                        out=xt_t[:],
                        in_=xt2[r * P : (r + 1) * P, base : base + 4 * L],
                    )
                xt_tiles[u] = xt_t

            emit_in(0, fine=True)
            for u in range(1, NU):
                emit_in(u)

            ptw_tiles = {}

            def emit_matvec(u):
                ptw = ptw_tp.tile([P, 512], F32, tag="ptw")
                xt_t = xt_tiles[u]
                for b in range(NB):
                    nc.tensor.matmul(
                        out=ptw[:],
                        lhsT=wrep_t[:, b * P : (b + 1) * P],
                        rhs=xt_t[:, b * 512 : (b + 1) * 512],
                        start=(b == 0),
                        stop=(b == NB - 1),
                    )
                ptw_tiles[u] = ptw

            emit_matvec(0)

            fv_tiles = {}
            for u in range(NU):
                r, h = divmod(u, 2)
                c0 = u * 4
                ptw = ptw_tiles.pop(u)
                xt_tiles.pop(u)
                psd = psd_tp.tile([P, 2 * PSUM_BANK], F32, tag="psd")
                # ---- tw extraction: psum row0 -> SBUF, 4 PE transposes ----
                twrow = twrow_tp.tile([1, 512], F32, tag="twrow")
                nc.scalar.copy(out=twrow[:], in_=ptw[0:1, :])
                for j in range(4):
                    nc.tensor.transpose(
                        out=psd[:, TW0 + j : TW0 + j + 1],
                        in_=twrow[0:1, j * P : (j + 1) * P],
                        identity=ones1_t[:],
                    )
                if u + 1 < NU:
                    emit_matvec(u + 1)  # PE fills the tw->rk dependency latency
                # relu(tw + b)
                tw_t = tw_tp.tile([P, 4], F32, tag="tw")
                nc.vector.tensor_scalar(
                    out=tw_t[:], in0=psd[:, TW0 : TW0 + 4],
                    scalar1=bcol_t[:, 0:1], scalar2=0.0,
                    op0=Op.add, op1=Op.max,
                )
                # ---- route one-hot for the 4 chunks in one op ----
                akh = akh_tp.tile([P, 4 * P], F16, tag="akh")
                nc.vector.tensor_tensor(
                    out=akh[:].rearrange("p (k q) -> p k q", q=P),
                    in0=ipt_t[:].rearrange("p (k q) -> p k q", q=P),
                    in1=route_t[:, c0 : c0 + 4].unsqueeze(2).broadcast_to((P, 4, P)),
                    op=Op.is_equal,
                )
                # ---- per-chunk rk + band/fixup matmuls ----
                for kk in range(4):
                    rk = rk_tp.tile([P, RKW], F16, tag="rk")
                    nc.vector.tensor_scalar(
                        out=rk[:], in0=iwc_t[:],
                        scalar1=offr_t[:, c0 + kk : c0 + kk + 1],
                        scalar2=tw_t[:, kk : kk + 1],
                        op0=Op.is_equal, op1=Op.mult,
                    )
                    lhs = akh[:, kk * P : (kk + 1) * P]
                    lo, hi = BANDS_H[h][kk]
                    nc.tensor.matmul(
                        out=psd[:, lo:hi],
                        lhsT=lhs,
                        rhs=rk[:, 0 : hi - lo],
                        start=True, stop=True,
                    )
                    nc.tensor.matmul(
                        out=psd[:, PSF0 + kk * MAXMEM : PSF0 + (kk + 1) * MAXMEM],
                        lhsT=lhs,
                        rhs=rk[:, FIXC0 : FIXC0 + MAXMEM],
                        start=True, stop=True,
                    )
                # ---- fixup values for this half ----
                fvh = fv_tp.tile([P, 1], F16, tag="fv")
                nc.vector.tensor_reduce(
                    out=fvh[:], in_=psd[:, PSF0 : PSF0 + 4 * MAXMEM],
                    axis=mybir.AxisListType.X, op=Op.max,
                )
                fv_tiles[u] = fvh
                # ---- dense evacuation + half writeback ----
                dense = dense_tp.tile([P, DW], F16, tag="dense")
                if u == NU - 1:
                    # last unit: evacuate on the (idle) DVE so it overlaps the
                    # previous unit's Act evacuation instead of queuing behind it
                    nc.vector.tensor_copy(out=dense[:], in_=psd[:, DOFF[h] : DOFF[h] + DW])
                else:
                    nc.scalar.copy(out=dense[:], in_=psd[:, DOFF[h] : DOFF[h] + DW])
                # late rows: issue from sync (idle after the input phase) so the
                # issue does not serialize behind the Act evacuation copies
                oeng = nc.gpsimd if r < 2 else nc.sync
                oeng.dma_start(
                    out=outs[r][:].rearrange("(p f) -> p f", f=W)[:, h * DW : (h + 1) * DW],
                    in_=dense[:],
                )
                if h == 1:
                    fixv = fixv_tp.tile([P, 1], F16, tag="fixv")
                    nc.vector.tensor_tensor(
                        out=fixv[:], in0=fv_tiles.pop(u - 1)[:], in1=fv_tiles.pop(u)[:],
                        op=Op.max,
                    )
                    nc.gpsimd.indirect_dma_start(
                        out=outs[r][:].unsqueeze(1),
                        out_offset=IndirectOffsetOnAxis(ap=fg_t[:, r : r + 1], axis=0),
                        in_=fixv[:, 0:1],
                        in_offset=None,
                        bounds_check=V - 1,
                        oob_is_err=False,
                    )

    _throttle_prefetch(nc)
    _split_excess_waits(nc)
    return nc


_prog_cache = {}


def _get_program():
    if "nc" not in _prog_cache:
        _prog_cache["nc"] = _build_program()
    return _prog_cache["nc"]


_BAND_OF = np.searchsorted(np.asarray(BB[1:]), np.arange(W), side="right")


def _make_in_maps(hidden_state, input_ids, w_sparse, b_sparse):
    hs = np.asarray(hidden_state, dtype=np.float32).reshape(B, L, H)
    ids_all = np.asarray(input_ids).astype(np.int64).reshape(B, L)
    w = np.asarray(w_sparse, dtype=np.float32).reshape(H)
    bval = float(np.asarray(b_sparse, dtype=np.float32).reshape(-1)[0])

    # constants shared by all cores
    wrep = np.ascontiguousarray(
        np.repeat(w.astype(np.float16).reshape(NB, P).T[:, :, None], P, axis=2)
        .reshape(P, NB * P)
    )
    ipt = np.broadcast_to(
        np.tile(np.arange(P, dtype=np.float16), 4), (P, 4 * P)
    ).copy()
    iwc_row = np.full(RKW, -5.0, np.float32)
    iwc_row[0:FIXC0] = np.arange(FIXC0, dtype=np.float32)
    iwc_row[FIXC0:] = MEMMARK + np.arange(MAXMEM, dtype=np.float32)
    iwc = np.broadcast_to(iwc_row, (P, RKW)).copy()
    bcol = np.full((P, 1), bval, np.float32)

    in_maps = []
    for c in range(NCORES):
        ids = ids_all[c * BS : (c + 1) * BS]
        hsc = hs[c * BS : (c + 1) * BS].reshape(BS * L, H)
        route = np.full((P, BS * NB), -1.0, np.float16)
        offrel = np.full((P, BS * NB), -1.0, np.float32)
        fixgid = np.full((P, BS), V, np.int32)
        perm = np.full((BS * L,), -1, np.int64)

        for r in range(BS):
            row = ids[r]
            vals, counts = np.unique(row, return_counts=True)
            cnt = dict(zip(vals.tolist(), counts.tolist()))
            slots = [0] * NB
            nclass = 0
            classmem = {}
            fixup_tokens = []
            for l in range(L):
                v = int(row[l])
                if v < 4:
                    continue
                p, f = divmod(v, W)
                k = int(_BAND_OF[f])
                if cnt[v] == 1 and slots[k] < P:
                    s = slots[k]
                    slots[k] += 1
                    perm[r * L + k * P + s] = r * L + l
                    route[s, r * NB + k] = p
                    offrel[s, r * NB + k] = f - BB[k]
                else:
                    fixup_tokens.append((l, v))
            kf = 0
            memcnt = {}
            for l, v in fixup_tokens:
                if v in classmem:
                    q = classmem[v]
                else:
                    q = nclass
                    nclass += 1
                    assert nclass <= MAXCLS, f"too many fixup classes: {nclass}"
                    fixgid[q, r] = v
                    classmem[v] = q
                while slots[kf] >= P:
                    kf += 1
                m = memcnt.get((q, kf), 0)
                assert m < MAXMEM, "fixup class larger than MAXMEM in one chunk"
                memcnt[(q, kf)] = m + 1
                s = slots[kf]
                slots[kf] += 1
                perm[r * L + kf * P + s] = r * L + l
                route[s, r * NB + kf] = q
                offrel[s, r * NB + kf] = MEMMARK + m

        tmp = np.zeros((BS * L, H), np.float16)
        valid = perm >= 0
        tmp[valid] = hsc[perm[valid]].astype(np.float16)
        # xt2[r*128 + p, h*4096 + b*512 + t'] = hidden_f16[token(r, h*512 + t'), b*128 + p]
        xt2 = np.ascontiguousarray(
            tmp.reshape(BS, 2, 512, NB, P).transpose(0, 4, 1, 3, 2).reshape(BS * P, NB * L)
        )
        in_maps.append(
            {
                "xt2": xt2,
                "wrep": wrep,
                "ipt": ipt,
                "iwc": iwc,
                "bcol": bcol,
                "route": route,
                "offrel": offrel,
                "fixgid": fixgid,
            }
        )
    return in_maps


def kernel(hidden_state, input_ids, w_sparse, b_sparse, _trace=False):
    nc = _get_program()
    in_maps = _make_in_maps(hidden_state, input_ids, w_sparse, b_sparse)
    res = run_bass_kernel_spmd(nc, in_maps, list(range(NCORES)), trace=_trace)
    out = np.empty((B, V), np.float32)
    for c in range(NCORES):
        for r in range(BS):
            out[c * BS + r] = np.asarray(res.results[c][f"out{r}"])[:V].astype(
                np.float32
            )
    if _trace:
        kernel.last_exec_time_ns = res.exec_time_ns
        kernel.last_results = res
    return out
